# revision 36
# baseline (speedup 1.0000x reference)
"""BinarizedLinear TRN2 kernel: y = x @ sign(weight).T + bias.

Full shapes: x [8192, 4096] f32, weight [4096, 4096] f32, bias [4096] f32
-> y [8192, 4096] f32.

Sharding across 8 NeuronCores: tokens split 2 ways x out_features split 4
ways. Each core computes a [4096, 1024] output block. The contraction is
mixed-precision: KDR=16 k-tiles run as fp8-e4m3 DoubleRow pair-matmuls
(two k-tiles per PE pass at double rate), the other 16 k-tiles run in
bf16. Binarized weights (+-1, exact in both formats) are produced
on-device by the ACT Sign LUT from a small f32 staging pool; x streams
in K-major strips cast f32->e4m3 / f32->bf16 by SWDGE cast-DMAs.
Phase A runs strips 0-3 k-major in lockstep with the weight stream (8
open PSUM banks consume each signed w tile on arrival, keeping the HAM
clock warm through the 24 MB input front); the remaining 28 strips run
group-serial with deep prefetch, and the last strip is split into
256-wide chains so the final eviction overlaps the tail matmuls.
TensorE accumulates everything in fp32 PSUM; bias is added on PSUM
eviction. The fp8 share is sized so the quantization error stays
~1.5e-2 max-rel, under the 2e-2 gate. Host does layout only (transpose/
tile/slice); sign, matmul and bias run on device.
"""
import sys

if "/opt/trn_rl_repo" not in sys.path:
    sys.path.insert(0, "/opt/trn_rl_repo")

import numpy as np
import concourse.bass as bass
import concourse.mybir as mybir
import concourse.tile as tile
from concourse.bass_utils import run_bass_kernel_spmd

TOKENS, IN_F, OUT_F = 8192, 4096, 4096
T_SHARDS, O_SHARDS = 2, 4
TOK_PER = TOKENS // T_SHARDS  # 4096 tokens per core
OUT_PER = OUT_F // O_SHARDS   # 1024 out features per core
P = 128
KT = IN_F // P                # 32 contraction tiles
TT = TOK_PER // P             # 32 token tiles
NH = OUT_PER // 512           # 2 psum-bank halves
XBUFS = 12                    # x strip prefetch depth
KDR = 16                      # k-tiles in fp8-e4m3 DoubleRow pairs
NDR = KDR // 2                # DoubleRow pair-matmuls per group
KBF = KT - KDR                # trailing k-tiles in bf16

F32 = mybir.dt.float32
BF16 = mybir.dt.bfloat16
FP8 = mybir.dt.float8e4
DR = mybir.MatmulPerfMode.DoubleRow


def split_excess_waits(nc, max_waits=1):
    """This walrus build encodes at most one semaphore wait per
    instruction; move excess waits onto preceding same-engine NoOps."""
    ctr = 0
    for fn in nc.m.functions:
        for bb in fn.blocks:
            insts = bb.instructions
            i = 0
            while i < len(insts):
                inst = insts[i]
                si = getattr(inst, "sync_info", None)
                ow = list(si.on_wait) if si else []
                if len(ow) > max_waits:
                    extra, keep = ow[:-max_waits], ow[-max_waits:]
                    si.on_wait = keep
                    inst.sync_info = si
                    k = 0
                    for j in range(0, len(extra), max_waits):
                        ctr += 1
                        nop = mybir.InstNoOp(
                            name=f"I-waitsplit-{ctr}", ins=[], outs=[]
                        )
                        nop.engine = inst.engine
                        nop.sync_info = mybir.SyncInfo(
                            on_wait=extra[j : j + max_waits], on_update=[]
                        )
                        insts.insert(i + k, nop)
                        k += 1
                    i += k
                i += 1
    return ctr


def build_nc():
    nc = bass.Bass()
    # xs: x shard pre-tiled on host to [TT, P(k_lo), KT*P(t-major)] so each
    # SBUF partition reads one contiguous 16 KB run per strip DMA.
    xs = nc.dram_tensor("xs", [TT, P, KT * P], F32, kind="ExternalInput")
    wT = nc.dram_tensor("wT", [IN_F, OUT_PER], F32, kind="ExternalInput")
    biasb = nc.dram_tensor("biasb", [P, OUT_PER], F32, kind="ExternalInput")
    y = nc.dram_tensor("y", [TOK_PER, OUT_PER], F32, kind="ExternalOutput")

    wT_r = wT.rearrange("(ko p) o -> p ko o", p=P)

    with tile.TileContext(nc) as tc:
        with (
            tc.tile_pool(name="wbin", bufs=1) as wbin_pool,
            tc.tile_pool(name="wstg", bufs=4) as wstg_pool,
            tc.tile_pool(name="xr", bufs=XBUFS) as xr_pool,
            tc.tile_pool(name="outp", bufs=4) as out_pool,
            tc.tile_pool(name="psum", bufs=8, space="PSUM") as psum_pool,
        ):
            def new_strip():
                # fp8 DoubleRow pairs + bf16 tail of one 128-token strip
                xdr = xr_pool.tile([P, NDR, 2, P], FP8, tag="xdr", name="xdr")
                xbf = xr_pool.tile([P, KBF, P], BF16, tag="xbf", name="xbf")
                return (xdr, xbf)

            def x_sub(xrpair, t, part):
                # SWDGE cast-DMAs: f32 DRAM -> fp8/bf16 SBUF (rounds).
                # part 0: k-tiles 0..KDR-1 -> xdr; 1/2: bf16 halves.
                xdr, xbf = xrpair
                if part == 0:
                    nc.gpsimd.dma_start(
                        xdr.rearrange("p a b t -> p (a b t)"),
                        xs[t, :, 0 : KDR * P],
                    )
                else:
                    h = KBF // 2
                    kk = (part - 1) * h
                    nc.gpsimd.dma_start(
                        xbf[:, kk : kk + h, :].rearrange("p k t -> p (k t)"),
                        xs[t, :, (KDR + kk) * P : (KDR + kk + h) * P],
                    )

            def load_x_strip(t):
                xrpair = new_strip()
                for part in range(3):
                    x_sub(xrpair, t, part)
                return xrpair

            pair_tiles = {}

            def sign_dst(k):
                # resident binarized tile slot for k-tile k; the fp8 pair
                # tile is shared by k-tiles 2p and 2p+1
                if k < KDR:
                    p, i = divmod(k, 2)
                    if p not in pair_tiles:
                        pair_tiles[p] = wbin_pool.tile(
                            [P, 2, OUT_PER], FP8, tag=f"wdr{p}", name=f"wdr{p}"
                        )
                    wb = pair_tiles[p]
                    return wb, (lambda sl: wb[:, i, sl])
                wb = wbin_pool.tile(
                    [P, OUT_PER], BF16, tag=f"wbf{k}", name=f"wbf{k}"
                )
                return wb, (lambda sl: wb[:, sl])

            def load_w(k, halves=False, via=None):
                # stage f32 tile, binarize via ACT Sign into resident
                # fp8 (DoubleRow pairs) or bf16 tiles; +-1 is exact in both
                stg = wstg_pool.tile([P, OUT_PER], F32, tag="wstg", name="stg")
                wb, dst = sign_dst(k)
                dma = (via or nc.gpsimd).dma_start
                if halves:
                    for h in range(2):
                        sl = slice(h * 512, (h + 1) * 512)
                        dma(stg[:, sl], wT_r[:, k, sl])
                        nc.scalar.sign(dst(sl), stg[:, sl])
                else:
                    dma(stg[:], wT_r[:, k, :])
                    for h in range(2):
                        sl = slice(h * 512, (h + 1) * 512)
                        nc.scalar.sign(dst(sl), stg[:, sl])
                return wb

            # bias via HWDGE on the sync queue: off the SWDGE FIFO, lands
            # in the first ~10us without displacing x/w bytes.
            bias_sb = wbin_pool.tile([P, OUT_PER], F32, tag="bias", name="bias")
            nc.sync.dma_start(bias_sb[:], biasb[:])

            # Phase A: strips 0-3 run K-MAJOR in lockstep with the weight
            # stream. 8 PSUM banks hold the 8 (t,oh) groups open across
            # the whole contraction; each w tile is consumed by 8 matmuls
            # the moment it is signed, so PE idle during the 24 MB front
            # stays in sub-HAM-window slivers and the clock never
            # re-throttles. Stream order: bf16 tiles first (one x sub-DMA
            # interleaved per tile, sections ahead of their deadlines),
            # fp8 DoubleRow pairs last, consumed pair-wise.
            w_order = list(range(KDR, KT)) + list(range(KDR))
            h0 = KBF // 2
            NA = 4  # phase-A strips

            x_strips = {t: new_strip() for t in range(NA)}
            x0 = x_strips[0]
            # first bf16 k-slice (64 KB) leads the SWDGE FIFO so the first
            # matmul issues as soon as the first w half-tile is signed
            nc.gpsimd.dma_start(
                x0[1][:, 0:1, :].rearrange("p k t -> p (k t)"),
                xs[0, :, KDR * P : (KDR + 1) * P],
            )
            w_slot_map = {}
            pss = {}
            mm_pos = [0]

            def phase_a_mms(pos):
                # k-major interleave across the 8 open banks for the bf16
                # stream section only; the fp8 DoubleRow section runs as
                # per-group serial tails (emitted after the stream) to
                # keep each bank's DR chain contiguous.
                k = w_order[pos]
                if k < KDR:
                    return
                idx = k - KDR
                for t in range(NA):
                    for oh in range(NH):
                        osl = slice(oh * 512, (oh + 1) * 512)
                        nc.tensor.matmul(
                            pss[(t, oh)][:],
                            x_strips[t][1][:, idx, :],
                            w_slot_map[k][:, osl],
                            start=(pos == 0),
                            stop=False,
                        )

            def emit_w(pos, halves=False):
                k = w_order[pos]
                w_slot_map[k] = load_w(k, halves=halves)

            for t in range(NA):
                for oh in range(NH):
                    pss[(t, oh)] = psum_pool.tile(
                        [P, 512], F32, tag="ps", name="ps"
                    )

            # interleaved FIFO schedule: x sections land ahead of the
            # stream position that first needs them
            emit_w(0, halves=True)
            nc.gpsimd.dma_start(
                x0[1][:, 1:h0, :].rearrange("p k t -> p (k t)"),
                xs[0, :, (KDR + 1) * P : (KDR + h0) * P],
            )
            x_sub(x_strips[1], 1, 1)
            emit_w(1, halves=True)
            x_sub(x_strips[2], 2, 1)
            x_sub(x_strips[3], 3, 1)
            emit_w(2)
            x_sub(x_strips[0], 0, 2)
            x_sub(x_strips[1], 1, 2)
            emit_w(3)
            x_sub(x_strips[2], 2, 2)
            x_sub(x_strips[3], 3, 2)
            emit_w(4)
            x_sub(x_strips[0], 0, 0)
            emit_w(5)
            x_sub(x_strips[1], 1, 0)
            emit_w(6)
            x_sub(x_strips[2], 2, 0)
            emit_w(7)
            x_sub(x_strips[3], 3, 0)
            for pos in range(8, KT):
                emit_w(pos)

            # phase-A matmuls are emitted AFTER every producer DMA/sign
            # above (a consumer emitted before its producer gets no
            # dependency edge); execution order is paced by the data
            # semaphores, not emission order.
            for pos in range(KT):
                phase_a_mms(pos)

            wdr = [w_slot_map[2 * p] for p in range(NDR)]
            wbf = [w_slot_map[KDR + kk] for kk in range(KBF)]

            # phase-A DR tails + evictions: each bank finishes its fp8
            # pair chain serially (pairs land at the stream tail at
            # ~2.6us each; 8 groups consume one pair in ~1.7us)
            for t in range(NA):
                for oh in range(NH):
                    osl = slice(oh * 512, (oh + 1) * 512)
                    for p in range(NDR):
                        nc.tensor.matmul(
                            pss[(t, oh)][:],
                            x_strips[t][0][:, p, :, :],
                            wdr[p][:, :, osl],
                            start=False,
                            stop=(p == NDR - 1),
                            perf_mode=DR,
                        )
                    out_sb = out_pool.tile([P, 512], F32, tag="out", name="out")
                    nc.vector.tensor_add(
                        out_sb[:], pss[(t, oh)][:], bias_sb[:, osl]
                    )
                    nc.sync.dma_start(y[t * P : (t + 1) * P, osl], out_sb[:])

            # phase-B prefetch queues behind the weight stream
            for t in range(NA, XBUFS):
                x_strips[t] = load_x_strip(t)

            def group_mms(ps, xrpair, osl, start_chain=True):
                xdr, xbf = xrpair
                for kk in range(KBF):
                    nc.tensor.matmul(
                        ps,
                        xbf[:, kk, :],
                        wbf[kk][:, osl],
                        start=(kk == 0),
                        stop=False,
                        skip_group_check=not start_chain,
                    )
                for p in range(NDR):
                    nc.tensor.matmul(
                        ps,
                        xdr[:, p, :, :],
                        wdr[p][:, :, osl],
                        start=False,
                        stop=(p == NDR - 1),
                        perf_mode=DR,
                        skip_group_check=not start_chain,
                    )

            for t in range(NA, TT):
                xrpair = x_strips.pop(t)
                if t + XBUFS - NA < TT:
                    x_strips[t + XBUFS - NA] = load_x_strip(t + XBUFS - NA)

                # Last strip: run each oh half as two sequential 256-wide
                # chains inside one PSUM bank so eviction + y-DMA of chain
                # i overlap chain i+1's matmuls, shrinking the exposed
                # kernel tail to a single 256-col eviction.
                chains = 2 if t == TT - 1 else 1
                cw = 512 // chains
                for oh in range(NH):
                    ps = psum_pool.tile([P, 512], F32, tag="ps", name="ps")
                    out_sb = out_pool.tile([P, 512], F32, tag="out", name="out")
                    for c in range(chains):
                        csl = slice(c * cw, (c + 1) * cw)
                        osl = slice(oh * 512 + c * cw, oh * 512 + (c + 1) * cw)
                        group_mms(
                            ps[:, csl], xrpair, osl, start_chain=(chains == 1)
                        )
                        nc.vector.tensor_add(
                            out_sb[:, csl], ps[:, csl], bias_sb[:, osl]
                        )
                        nc.sync.dma_start(
                            y[t * P : (t + 1) * P, osl], out_sb[:, csl]
                        )

    split_excess_waits(nc)
    return nc


_NC = None


def _get_nc():
    global _NC
    if _NC is None:
        _NC = build_nc()
    return _NC


def make_in_maps(x, weight, bias):
    x = np.asarray(x, dtype=np.float32)
    weight = np.asarray(weight, dtype=np.float32)
    bias = np.asarray(bias, dtype=np.float32)
    wT = np.ascontiguousarray(weight.T)  # [IN_F, OUT_F]
    in_maps = []
    for c in range(8):
        th, oq = divmod(c, O_SHARDS)
        xsh = x[th * TOK_PER : (th + 1) * TOK_PER]  # [TOK_PER, IN_F]
        # [TT, P_t, KT, P_k] -> [TT, P_k, KT, P_t]: partition dim = k_lo,
        # contiguous 16 KB per partition per strip
        xt = np.ascontiguousarray(
            xsh.reshape(TT, P, KT, P).transpose(0, 3, 2, 1)
        ).reshape(TT, P, KT * P)
        in_maps.append(
            {
                "xs": xt,
                "wT": np.ascontiguousarray(
                    wT[:, oq * OUT_PER : (oq + 1) * OUT_PER]
                ),
                "biasb": np.ascontiguousarray(
                    np.broadcast_to(
                        bias[oq * OUT_PER : (oq + 1) * OUT_PER], (P, OUT_PER)
                    )
                ),
            }
        )
    return in_maps


def assemble(results):
    out = np.empty((TOKENS, OUT_F), np.float32)
    for c in range(8):
        th, oq = divmod(c, O_SHARDS)
        out[
            th * TOK_PER : (th + 1) * TOK_PER,
            oq * OUT_PER : (oq + 1) * OUT_PER,
        ] = results[c]["y"]
    return out


def kernel(x, weight, bias):
    in_maps = make_in_maps(x, weight, bias)
    res = run_bass_kernel_spmd(_get_nc(), in_maps, core_ids=list(range(8)))
    return assemble(res.results)
